# revision 7
# baseline (speedup 1.0000x reference)
"""CvT attention block (depthwise conv QKV projections + MHA) on 8 Trainium2
NeuronCores, data-parallel over batch.

Per core (one batch image), everything SBUF-resident, f32r matmuls:
  A) depthwise 3x3 convs (stride 1 for Q; stride 2 for K/V) as 9 shifted
     diag-matmuls per 128-channel chunk + edge-wrap fixups; BN folded into
     tap weights (scale) and the ACT eviction (bias).
  B) projections: Q^T = Wq^T y_q  [co, l] layout; K^T likewise; V-hat = y_v Wv
     in [t, co] layout with a ones-column per head (gives softmax denominators
     for free in the AV matmul).
  C) per (l-chunk, head): scores S^T = K^T(head)^T Q^T(head) -> exp on ACT ->
     AV accumulate (65 rows: 64 of O^T + 1 of sums) -> reciprocal; then
     normalization broadcast via indicator matmul + output projection + bias.
"""

import numpy as np
from concourse import mybir
import concourse.bacc as bacc
import concourse.tile as tile
from concourse.bass_utils import run_bass_kernel_spmd

F32 = mybir.dt.float32
F32R = mybir.dt.float32r
AFT = mybir.ActivationFunctionType

C = 384
HW = 56
T = 3136            # 56*56
TKV = 784           # 28*28
NH = 6
D = 64
SCALE = C ** (-0.5)
EPS = 1e-5
XP = 57 + T + 57    # padded x row length per channel

L_CHUNKS = [(i * 512, min(512, T - i * 512)) for i in range((T + 511) // 512)]
T_TILES = [(i * 128, min(128, TKV - i * 128)) for i in range((TKV + 127) // 128)]
KV_CHUNKS = [(0, 512), (512, 272)]

_CACHE = {}


def _emit(nc, tc, ctx, d, reps):
    import contextlib

    pers = ctx.enter_context(tc.tile_pool(name="pers", bufs=1))

    # ---- persistent tiles ----
    wq = [pers.tile([128, C], F32R, tag=f"wq{i}", name=f"wq{i}") for i in range(3)]
    wk = [pers.tile([128, C], F32R, tag=f"wk{i}", name=f"wk{i}") for i in range(3)]
    wvp = [pers.tile([128, NH * 65], F32R, tag=f"wvp{i}", name=f"wvp{i}") for i in range(3)]
    wpj = [pers.tile([128, C], F32R, tag=f"wpj{i}", name=f"wpj{i}") for i in range(3)]
    ind6 = [pers.tile([6, 128], F32R, tag=f"ind6{i}", name=f"ind6{i}") for i in range(3)]
    bq = [pers.tile([128, 1], F32, tag=f"bq{i}", name=f"bq{i}") for i in range(3)]
    bk = [pers.tile([128, 1], F32, tag=f"bk{i}", name=f"bk{i}") for i in range(3)]
    bv = [pers.tile([128, 1], F32, tag=f"bv{i}", name=f"bv{i}") for i in range(3)]
    bpj = pers.tile([128, C], F32, tag="bpj", name="bpj")
    QT = [pers.tile([128, T], F32R, tag=f"QT{i}", name=f"QT{i}") for i in range(3)]
    KT = [pers.tile([128, TKV], F32R, tag=f"KT{i}", name=f"KT{i}") for i in range(3)]
    Vh = [pers.tile([128, NH * 65], F32R, tag=f"Vh{i}", name=f"Vh{i}") for i in range(len(T_TILES))]

    for i in range(3):
        nc.sync.dma_start(wq[i][:], d["wq"][i * 128:(i + 1) * 128, :])
        nc.sync.dma_start(wk[i][:], d["wk"][i * 128:(i + 1) * 128, :])
        nc.sync.dma_start(wvp[i][:], d["wvp"][i * 128:(i + 1) * 128, :])
        nc.sync.dma_start(wpj[i][:], d["wpj"][i * 128:(i + 1) * 128, :])
        nc.sync.dma_start(ind6[i][:], d["ind6"][i])
        nc.sync.dma_start(bq[i][:], d["bq"][i])
        nc.sync.dma_start(bk[i][:], d["bk"][i])
        nc.sync.dma_start(bv[i][:], d["bv"][i])
    nc.sync.dma_start(bpj[:], d["bpj"])

    for rep in range(reps):
        sfx = f"r{rep}"
        with contextlib.ExitStack() as phAB:
            ypool = phAB.enter_context(tc.tile_pool(name="y" + sfx, bufs=1))
            yq = [ypool.tile([128, T], F32R, tag=f"yq{i}", name=f"yq{i}") for i in range(3)]
            yk = [ypool.tile([128, TKV], F32R, tag=f"yk{i}", name=f"yk{i}") for i in range(3)]
            yv = [ypool.tile([128, TKV], F32R, tag=f"yv{i}", name=f"yv{i}") for i in range(3)]

            # ---------------- Phase A: depthwise conv + BN ----------------
            with contextlib.ExitStack() as phA:
                xpool = phA.enter_context(tc.tile_pool(name="x" + sfx, bufs=2))
                dpool = phA.enter_context(tc.tile_pool(name="dg" + sfx, bufs=2))
                psA = phA.enter_context(
                    tc.tile_pool(name="psA" + sfx, bufs=3, space="PSUM"))
                psF = phA.enter_context(
                    tc.tile_pool(name="psF" + sfx, bufs=2, space="PSUM"))

                for ch in range(3):
                    xt = xpool.tile([128, XP], F32R, tag="x", name="x")
                    nc.sync.dma_start(xt[:], d["xp"][ch * 128:(ch + 1) * 128, :])

                    # --- Q conv: stride 1, 7 blocks of 8 output rows ---
                    dg = dpool.tile([128, 9, 128], F32R, tag="dg", name="dg")
                    nc.sync.dma_start(dg[:], d["dg"][0, ch])
                    for blk in range(7):
                        r0 = blk * 8
                        p = psA.tile([128, 448], F32, tag="psA", name="psA")
                        for t in range(9):
                            di, dj = t // 3 - 1, t % 3 - 1
                            base = 57 + (r0 + di) * 56 + dj
                            nc.tensor.matmul(
                                p[:], dg[:, t, :], xt[:, base:base + 448],
                                start=(t == 0), stop=(t == 8))
                        # edge-wrap fixups: cols 0-7 left err, 8-15 right err
                        pf = psF.tile([128, 16], F32, tag="psF", name="psF")
                        for k, di in enumerate((-1, 0, 1)):
                            bl = 57 + (r0 + di - 1) * 56 + 55
                            nc.tensor.matmul(
                                pf[:, 0:8], dg[:, 3 * (di + 1), :],
                                xt[:, bl:bl + 393:56],
                                start=(k == 0), stop=False)
                        for k, di in enumerate((-1, 0, 1)):
                            br = 57 + (r0 + di + 1) * 56
                            nc.tensor.matmul(
                                pf[:, 8:16], dg[:, 3 * (di + 1) + 2, :],
                                xt[:, br:br + 393:56],
                                start=False, stop=(k == 2))
                        yb = yq[ch][:, r0 * 56:(r0 + 8) * 56]
                        nc.scalar.activation(yb, p[:], AFT.Identity,
                                             bias=bq[ch][:], scale=1.0)
                        yl = yb[:, 0:393:56]
                        yr = yb[:, 55:448:56]
                        nc.vector.tensor_sub(yl, yl, pf[:, 0:8])
                        nc.vector.tensor_sub(yr, yr, pf[:, 8:16])

                    # --- K and V convs: stride 2, 2 blocks of 14 rows ---
                    for cv, (yt, bt) in enumerate(((yk, bk), (yv, bv))):
                        dg = dpool.tile([128, 9, 128], F32R, tag="dg", name="dg")
                        nc.sync.dma_start(dg[:], d["dg"][cv + 1, ch])
                        for blk in range(2):
                            i0 = blk * 14
                            p = psA.tile([128, 392], F32, tag="psA", name="psA")
                            for t in range(9):
                                di, dj = t // 3 - 1, t % 3 - 1
                                base = 57 + (2 * i0 + di) * 56 + dj
                                rhs = xt[:, base:base + 1568].rearrange(
                                    "p (r q) -> p r q", q=112)[:, :, 0:56:2]
                                nc.tensor.matmul(p[:], dg[:, t, :], rhs,
                                                 start=(t == 0), stop=(t == 8))
                            pf = psF.tile([128, 16], F32, tag="psF", name="psF")
                            for k, di in enumerate((-1, 0, 1)):
                                bl = 57 + (2 * i0 + di - 1) * 56 + 55
                                nc.tensor.matmul(
                                    pf[:, 0:14], dg[:, 3 * (di + 1), :],
                                    xt[:, bl:bl + 1457:112],
                                    start=(k == 0), stop=(k == 2))
                            yb = yt[ch][:, i0 * 28:(i0 + 14) * 28]
                            nc.scalar.activation(yb, p[:], AFT.Identity,
                                                 bias=bt[ch][:], scale=1.0)
                            yl = yb[:, 0:365:28]
                            nc.vector.tensor_sub(yl, yl, pf[:, 0:14])

            # ---------------- Phase B: Q^T / K^T / V-hat projections -------
            with contextlib.ExitStack() as phB:
                psB = phB.enter_context(
                    tc.tile_pool(name="psB" + sfx, bufs=4, space="PSUM"))
                for co in range(3):
                    for lo, ls in L_CHUNKS:
                        p = psB.tile([128, 512], F32, tag="psB", name="psB")
                        for ch in range(3):
                            nc.tensor.matmul(
                                p[:, :ls], wq[ch][:, co * 128:(co + 1) * 128],
                                yq[ch][:, lo:lo + ls],
                                start=(ch == 0), stop=(ch == 2))
                        nc.vector.tensor_copy(QT[co][:, lo:lo + ls], p[:, :ls])
                    for lo, ls in KV_CHUNKS:
                        p = psB.tile([128, 512], F32, tag="psB", name="psB")
                        for ch in range(3):
                            nc.tensor.matmul(
                                p[:, :ls], wk[ch][:, co * 128:(co + 1) * 128],
                                yk[ch][:, lo:lo + ls],
                                start=(ch == 0), stop=(ch == 2))
                        nc.vector.tensor_copy(KT[co][:, lo:lo + ls], p[:, :ls])
                for ti, (to, ts) in enumerate(T_TILES):
                    p = psB.tile([128, NH * 65], F32, tag="psB", name="psB")
                    for ch in range(3):
                        nc.tensor.matmul(p[0:ts, :], yv[ch][:, to:to + ts],
                                         wvp[ch][:], start=(ch == 0),
                                         stop=(ch == 2))
                    nc.vector.tensor_copy(Vh[ti][0:ts, :], p[0:ts, :])
                    nc.vector.memset(Vh[ti][0:ts, 64:NH * 65:65].bitcast(F32), 1.0)

        # ------- Phase C: attention + normalization + out-proj, per l ------
        with contextlib.ExitStack() as phC:
            etp = phC.enter_context(tc.tile_pool(name="et" + sfx, bufs=10))
            otp = phC.enter_context(tc.tile_pool(name="ot" + sfx, bufs=2))
            rcp = phC.enter_context(tc.tile_pool(name="rc" + sfx, bufs=3))
            outp = phC.enter_context(tc.tile_pool(name="out" + sfx, bufs=3))
            psS = phC.enter_context(
                tc.tile_pool(name="psS" + sfx, bufs=3, space="PSUM"))
            psO = phC.enter_context(
                tc.tile_pool(name="psO" + sfx, bufs=2, space="PSUM"))
            psR = phC.enter_context(
                tc.tile_pool(name="psR" + sfx, bufs=1, space="PSUM"))
            psU = phC.enter_context(
                tc.tile_pool(name="psU" + sfx, bufs=2, space="PSUM"))

            for lo, ls in L_CHUNKS:
                OT = [otp.tile([128, 512], F32R, tag=f"ot{i}", name=f"ot{i}") for i in range(3)]
                rc6 = rcp.tile([6, 512], F32, tag="rc6", name="rc6")
                rc6r = rcp.tile([6, 512], F32R, tag="rc6r", name="rc6r")
                for h in range(NH):
                    c2, po = h // 2, 64 * (h % 2)
                    ets = []
                    for ti, (to, ts) in enumerate(T_TILES):
                        p = psS.tile([128, 512], F32, tag="psS", name="psS")
                        nc.tensor.matmul(p[0:ts, 0:ls],
                                         KT[c2][po:po + 64, to:to + ts],
                                         QT[c2][po:po + 64, lo:lo + ls],
                                         start=True, stop=True)
                        et = etp.tile([128, 512], F32R, tag="et", name="et")
                        nc.scalar.activation(et[0:ts, 0:ls], p[0:ts, 0:ls],
                                             AFT.Exp, scale=float(SCALE))
                        ets.append(et)
                    po2 = psO.tile([65, 512], F32, tag="psO", name="psO")
                    for ti, (to, ts) in enumerate(T_TILES):
                        nc.tensor.matmul(po2[:, :ls],
                                         Vh[ti][0:ts, h * 65:(h + 1) * 65],
                                         ets[ti][0:ts, 0:ls],
                                         start=(ti == 0),
                                         stop=(ti == len(T_TILES) - 1))
                    nc.vector.tensor_copy(OT[c2][po:po + 64, :ls],
                                          po2[0:64, :ls])
                    rtmp = rcp.tile([1, 512], F32, tag="rtmp", name="rtmp")
                    nc.vector.tensor_copy(rtmp[0:1, :ls], po2[64:65, :ls])
                    nc.sync.dma_start(rc6[h:h + 1, :ls], rtmp[0:1, :ls])

                with nc.allow_low_precision(reason="f32r recip for rb matmul"):
                    nc.vector.reciprocal(rc6r[0:6, :ls], rc6[0:6, :ls])
                for ch in range(3):
                    p = psR.tile([128, 512], F32, tag="psR", name="psR")
                    nc.tensor.matmul(p[:, :ls], ind6[ch][:], rc6r[0:6, :ls],
                                     start=True, stop=True)
                    nc.vector.tensor_mul(OT[ch][:, :ls], OT[ch][:, :ls],
                                         p[:, :ls])
                for j in range(0, ls, 128):
                    lsz = min(128, ls - j)
                    p = psU.tile([128, C], F32, tag="psU", name="psU")
                    for ch in range(3):
                        nc.tensor.matmul(p[0:lsz, :], OT[ch][:, j:j + lsz],
                                         wpj[ch][:], start=(ch == 0),
                                         stop=(ch == 2))
                    ot = outp.tile([128, C], F32, tag="o", name="o")
                    nc.vector.tensor_add(ot[0:lsz, :], p[0:lsz, :], bpj[0:lsz, :])
                    nc.sync.dma_start(d["out"][lo + j:lo + j + lsz, :],
                                      ot[0:lsz, :])


def _build(reps=1):
    key = reps
    if key in _CACHE:
        return _CACHE[key]
    import contextlib

    nc = bacc.Bacc("TRN2", target_bir_lowering=False, debug=False)
    d = {
        "xp": nc.dram_tensor("xp", [C, XP], F32R, kind="ExternalInput").ap(),
        "dg": nc.dram_tensor("dg", [3, 3, 128, 9, 128], F32R,
                             kind="ExternalInput").ap(),
        "wq": nc.dram_tensor("wq", [C, C], F32R, kind="ExternalInput").ap(),
        "wk": nc.dram_tensor("wk", [C, C], F32R, kind="ExternalInput").ap(),
        "wvp": nc.dram_tensor("wvp", [C, NH * 65], F32R,
                              kind="ExternalInput").ap(),
        "wpj": nc.dram_tensor("wpj", [C, C], F32R, kind="ExternalInput").ap(),
        "ind6": nc.dram_tensor("ind6", [3, 6, 128], F32R,
                               kind="ExternalInput").ap(),
        "bq": nc.dram_tensor("bq", [3, 128, 1], F32, kind="ExternalInput").ap(),
        "bk": nc.dram_tensor("bk", [3, 128, 1], F32, kind="ExternalInput").ap(),
        "bv": nc.dram_tensor("bv", [3, 128, 1], F32, kind="ExternalInput").ap(),
        "bpj": nc.dram_tensor("bpj", [128, C], F32, kind="ExternalInput").ap(),
        "out": nc.dram_tensor("out", [T, C], F32, kind="ExternalOutput").ap(),
    }
    with tile.TileContext(nc) as tc:
        with contextlib.ExitStack() as ctx:
            _emit(nc, tc, ctx, d, reps)
    nc.compile()
    _CACHE[key] = nc
    return nc


def _host_prep(x, conv_q, conv_k, conv_v, bn_q, bn_k, bn_v, Wq, Wk, Wv,
               Wproj, bproj):
    B = x.shape[0]
    x = np.asarray(x, np.float32)
    # x: [B, T, C] -> padded channel-major [B, C, XP]
    xp = np.zeros((B, C, XP), np.float32)
    xp[:, :, 57:57 + T] = np.ascontiguousarray(x.transpose(0, 2, 1))

    dg = np.zeros((3, 3, 128, 9, 128), np.float32)
    biases = []
    for cv, (w, bn) in enumerate(((conv_q, bn_q), (conv_k, bn_k),
                                  (conv_v, bn_v))):
        g, b, m, v = [np.asarray(bn[i], np.float64) for i in range(4)]
        a = g / np.sqrt(v + EPS)
        bias = (b - m * a).astype(np.float32)
        wh = (np.asarray(w, np.float64).reshape(C, 9) * a[:, None]).astype(
            np.float32)
        for ch in range(3):
            for t in range(9):
                dg[cv, ch, np.arange(128), t, np.arange(128)] = \
                    wh[ch * 128:(ch + 1) * 128, t]
        biases.append(bias.reshape(3, 128, 1))

    wvp = np.zeros((C, NH * 65), np.float32)
    Wv = np.asarray(Wv, np.float32)
    for h in range(NH):
        wvp[:, h * 65:h * 65 + 64] = Wv[:, h * 64:(h + 1) * 64]

    ind6 = np.zeros((3, 6, 128), np.float32)
    for ch in range(3):
        ind6[ch, 2 * ch, 0:64] = 1.0
        ind6[ch, 2 * ch + 1, 64:128] = 1.0

    return {
        "xp": xp,
        "dg": dg,
        "wq": np.asarray(Wq, np.float32),
        "wk": np.asarray(Wk, np.float32),
        "wvp": wvp,
        "wpj": np.asarray(Wproj, np.float32),
        "ind6": ind6,
        "bq": biases[0], "bk": biases[1], "bv": biases[2],
        "bpj": np.tile(np.asarray(bproj, np.float32)[None, :], (128, 1)),
    }


def kernel(x, h, w, conv_q, conv_k, conv_v, bn_q, bn_k, bn_v, Wq, Wk, Wv,
           Wproj, bproj, _reps=1, _nc=None):
    B = x.shape[0]
    nc = _nc if _nc is not None else _build(_reps)
    hp = _host_prep(x, conv_q, conv_k, conv_v, bn_q, bn_k, bn_v, Wq, Wk, Wv,
                    Wproj, bproj)
    shared = {k: v for k, v in hp.items() if k != "xp"}
    in_maps = [dict(shared, xp=hp["xp"][b]) for b in range(B)]
    res = run_bass_kernel_spmd(nc, in_maps, core_ids=list(range(B)))
    out = np.stack([res.results[b]["out"] for b in range(B)], axis=0)
    return out.astype(np.float32)


# revision 13
# speedup vs baseline: 1.2326x; 1.2326x over previous
"""CvT attention block (depthwise conv QKV + MHA) on 8 Trainium2 NeuronCores,
data-parallel over batch. Instruction-count-minimized variant:

  A) depthwise 3x3 convs on DVE: one fused per-partition-scalar FMA
     (scalar_tensor_tensor) per tap with edge-restricted access patterns,
     f32 accumulation scratch, single convert to bf16.
  B) projections in bf16 (moving dim up to 1024): Q^T/K^T in [co, l] layout,
     V-hat in [t, co] layout with a ones column per head (softmax denominators
     fall out of the AV matmul for free).
  C) per (l-chunk of 1024, head): S^T = K_h Q_h^T via PE (two score tiles
     packed in one 4-bank PSUM tile), one Exp over the pair on ACT, AV
     accumulation; then reciprocal + indicator-matmul broadcast for the
     softmax normalization, and the output projection in [co, l] layout with
     a transposing DMA store.
"""

import contextlib
import numpy as np
import ml_dtypes
from concourse import mybir
import concourse.bacc as bacc
import concourse.tile as tile
from concourse.bass_utils import run_bass_kernel_spmd

F32 = mybir.dt.float32
F32R = mybir.dt.float32r
BF16 = mybir.dt.bfloat16
AFT = mybir.ActivationFunctionType
ALU = mybir.AluOpType

C = 384
T = 3136            # 56*56
TKV = 784           # 28*28
NH = 6
SCALE = C ** (-0.5)
EPS = 1e-5
XP = 56 + T + 56

LC = [(i * 512, min(512, T - i * 512)) for i in range(7)]
T_TILES = [(i * 128, min(128, TKV - i * 128)) for i in range(7)]

_CACHE = {}


def _conv(nc, xt, ys, ybf, wb, cv, ch, stride):
    """Depthwise 3x3 conv for one 128-channel chunk on DVE.
    xt: [128, XP] f32 padded input (row r col c of the image lives at flat
    56 + r*56 + c, i.e. x3[1+r, c]).
    ys: f32 scratch [128, out_pix]; ybf: bf16 destination.
    wb: [128, 30] tile; tap t of conv cv at col 9*cv+t, bias at col 27+cv.
    """
    w = lambda t: wb[:, 9 * cv + t:9 * cv + t + 1]
    bias = wb[:, 27 + cv:28 + cv]
    x3 = xt[:, 0:3248].rearrange("p (r c) -> p r c", c=56)  # rows -1..56
    if stride == 1:
        ys3 = ys[:].rearrange("p (r c) -> p r c", c=56)
        # seed with center tap + bias
        nc.vector.tensor_scalar(ys[:], xt[:, 56:56 + T], w(4), bias,
                                op0=ALU.mult, op1=ALU.add)
        for t in (0, 1, 2, 3, 5, 6, 7, 8):
            di, dj = t // 3 - 1, t % 3 - 1
            if dj == 0:
                o = ys[:]
                i = xt[:, 56 + 56 * di:56 + 56 * di + T]
            elif dj < 0:
                o = ys3[:, :, 1:56]
                i = x3[:, 1 + di:57 + di, 0:55]
            else:
                o = ys3[:, :, 0:55]
                i = x3[:, 1 + di:57 + di, 1:56]
            nc.vector.scalar_tensor_tensor(o, i, w(t), o, op0=ALU.mult,
                                           op1=ALU.add)
        nc.vector.tensor_copy(ybf[:], ys[:])
    else:
        ysv = ys[:, 0:TKV]
        ys3 = ysv.rearrange("p (r c) -> p r c", c=28)
        nc.vector.tensor_scalar(ysv, x3[:, 1:57:2, 0:56:2], w(4), bias,
                                op0=ALU.mult, op1=ALU.add)
        for t in (0, 1, 2, 3, 5, 6, 7, 8):
            di, dj = t // 3 - 1, t % 3 - 1
            if dj == 0:
                o = ysv
                i = x3[:, 1 + di:57 + di:2, 0:56:2]
            elif dj < 0:
                o = ys3[:, :, 1:28]
                i = x3[:, 1 + di:57 + di:2, 1:54:2]
            else:
                o = ysv
                i = x3[:, 1 + di:57 + di:2, 1:56:2]
            nc.vector.scalar_tensor_tensor(o, i, w(t), o, op0=ALU.mult,
                                           op1=ALU.add)
        nc.vector.tensor_copy(ybf[:], ysv)


def _emit(nc, tc, ctx, d, reps):
    pers = ctx.enter_context(tc.tile_pool(name="pers", bufs=1))

    wq = [pers.tile([128, C], BF16, tag=f"wq{i}", name=f"wq{i}") for i in range(3)]
    wk = [pers.tile([128, C], BF16, tag=f"wk{i}", name=f"wk{i}") for i in range(3)]
    wvp = [pers.tile([128, NH * 65], BF16, tag=f"wvp{i}", name=f"wvp{i}")
           for i in range(3)]
    wpj = [pers.tile([128, C], BF16, tag=f"wpj{i}", name=f"wpj{i}")
           for i in range(3)]
    ind6 = [pers.tile([6, 128], F32R, tag=f"ind6{i}", name=f"ind6{i}")
            for i in range(3)]
    wb = [pers.tile([128, 30], F32, tag=f"wb{i}", name=f"wb{i}")
          for i in range(3)]
    bpjR = pers.tile([1, C], F32R, tag="bpjR", name="bpjR")
    ones1 = pers.tile([1, 512], F32R, tag="ones1", name="ones1")
    QT = [pers.tile([128, T], BF16, tag=f"QT{i}", name=f"QT{i}") for i in range(3)]
    KT = [pers.tile([128, TKV], BF16, tag=f"KT{i}", name=f"KT{i}")
          for i in range(3)]
    Vh = [pers.tile([128, NH * 65], BF16, tag=f"Vh{i}", name=f"Vh{i}")
          for i in range(7)]

    for i in range(3):
        nc.sync.dma_start(wq[i][:], d["wq"][i * 128:(i + 1) * 128, :])
        nc.sync.dma_start(wk[i][:], d["wk"][i * 128:(i + 1) * 128, :])
        nc.sync.dma_start(wvp[i][:], d["wvp"][i * 128:(i + 1) * 128, :])
        nc.sync.dma_start(wpj[i][:], d["wpj"][i * 128:(i + 1) * 128, :])
        nc.sync.dma_start(ind6[i][:], d["ind6"][i])
        nc.sync.dma_start(wb[i][:], d["wb"][i])
    nc.sync.dma_start(bpjR[:], d["bpjR"])
    nc.vector.memset(ones1[:].bitcast(F32), 1.0)

    for rep in range(reps):
        sfx = f"r{rep}"
        with contextlib.ExitStack() as phAB:
            ypool = phAB.enter_context(tc.tile_pool(name="y" + sfx, bufs=1))
            yq = [ypool.tile([128, T], BF16, tag=f"yq{i}", name=f"yq{i}")
                  for i in range(3)]
            yk = [ypool.tile([128, TKV], BF16, tag=f"yk{i}", name=f"yk{i}")
                  for i in range(3)]
            yv = [ypool.tile([128, TKV], BF16, tag=f"yv{i}", name=f"yv{i}")
                  for i in range(3)]

            # ---- Phase A: depthwise convs on DVE ----
            with contextlib.ExitStack() as phA:
                xpool = phA.enter_context(tc.tile_pool(name="x" + sfx, bufs=2))
                spool = phA.enter_context(tc.tile_pool(name="ys" + sfx, bufs=2))
                for ch in range(3):
                    xt = xpool.tile([128, XP], F32, tag="x", name="x")
                    nc.sync.dma_start(xt[:], d["xp"][ch * 128:(ch + 1) * 128, :])
                    ys = spool.tile([128, T], F32, tag="ys", name="ys")
                    _conv(nc, xt, ys, yq[ch], wb[ch], 0, ch, 1)
                    ys = spool.tile([128, T], F32, tag="ys", name="ys")
                    _conv(nc, xt, ys, yk[ch], wb[ch], 1, ch, 2)
                    ys = spool.tile([128, T], F32, tag="ys", name="ys")
                    _conv(nc, xt, ys, yv[ch], wb[ch], 2, ch, 2)

            # ---- Phase B: projections (bf16, 512-wide windows) ----
            with contextlib.ExitStack() as phB:
                psB = phB.enter_context(
                    tc.tile_pool(name="psB" + sfx, bufs=2, space="PSUM"))
                for co in range(3):
                    for g, grp in enumerate((LC[0:4], LC[4:7])):
                        p = psB.tile([128, 2048], F32, tag="psB", name="psB")
                        for k, (lo, ls) in enumerate(grp):
                            for ch in range(3):
                                nc.tensor.matmul(
                                    p[0:128, k * 512:k * 512 + ls],
                                    wq[ch][:, co * 128:(co + 1) * 128],
                                    yq[ch][:, lo:lo + ls],
                                    start=(ch == 0), stop=(ch == 2))
                        base = grp[0][0]
                        wid = grp[-1][0] + grp[-1][1] - base
                        nc.vector.tensor_copy(QT[co][:, base:base + wid],
                                              p[:, 0:wid])
                    p = psB.tile([128, 2048], F32, tag="psB", name="psB")
                    for k, (to, ts) in enumerate(((0, 512), (512, 272))):
                        for ch in range(3):
                            nc.tensor.matmul(
                                p[:, k * 512:k * 512 + ts],
                                wk[ch][:, co * 128:(co + 1) * 128],
                                yk[ch][:, to:to + ts],
                                start=(ch == 0), stop=(ch == 2))
                    nc.vector.tensor_copy(KT[co][:], p[:, 0:TKV])
                for gi in range(2):
                    tt = T_TILES[4 * gi:4 * gi + 4]
                    p = psB.tile([128, 2048], F32, tag="psB", name="psB")
                    for k, (to, ts) in enumerate(tt):
                        for ch in range(3):
                            nc.tensor.matmul(
                                p[0:ts, k * 512:k * 512 + NH * 65],
                                yv[ch][:, to:to + ts], wvp[ch][:],
                                start=(ch == 0), stop=(ch == 2))
                    for k, (to, ts) in enumerate(tt):
                        ti = 4 * gi + k
                        nc.vector.tensor_copy(
                            Vh[ti][0:ts, :], p[0:ts, k * 512:k * 512 + NH * 65])
                        nc.vector.memset(Vh[ti][0:ts, 64:NH * 65:65], 1.0)

            if "dbg_yq" in d:
                for i in range(3):
                    nc.sync.dma_start(d["dbg_yq"][i], yq[i][:])
                    nc.sync.dma_start(d["dbg_yk"][i], yk[i][:])
                    nc.sync.dma_start(d["dbg_qt"][i], QT[i][:])
                    nc.sync.dma_start(d["dbg_kt"][i], KT[i][:])
                for i in range(7):
                    nc.sync.dma_start(d["dbg_vh"][i], Vh[i][:])

        # ---- Phase C: attention + normalize + output projection ----
        with contextlib.ExitStack() as phC:
            etp = phC.enter_context(tc.tile_pool(name="et" + sfx, bufs=4))
            otp = phC.enter_context(tc.tile_pool(name="ot" + sfx, bufs=2))
            rcp = phC.enter_context(tc.tile_pool(name="rc" + sfx, bufs=2))
            outp = phC.enter_context(tc.tile_pool(name="ou" + sfx, bufs=2))
            psS = phC.enter_context(
                tc.tile_pool(name="psS" + sfx, bufs=1, space="PSUM"))
            psO = phC.enter_context(
                tc.tile_pool(name="psO" + sfx, bufs=2, space="PSUM"))

            for lo, ls in LC:
                OTb = otp.tile([128, 1536], BF16, tag="otb", name="otb")
                rcf = rcp.tile([1, NH * 512], F32, tag="rcf", name="rcf")
                rc6 = rcp.tile([6, 512], F32, tag="rc6", name="rc6")
                rc6r = rcp.tile([6, 512], F32R, tag="rc6r", name="rc6r")
                for h in range(NH):
                    c2, po = h // 2, 64 * (h % 2)
                    ets = []
                    for gi, tt in enumerate((T_TILES[0:4], T_TILES[4:7])):
                        p = psS.tile([128, 2048], F32, tag="psS", name="psS")
                        for k, (to, ts) in enumerate(tt):
                            nc.tensor.matmul(p[0:ts, k * 512:k * 512 + ls],
                                             KT[c2][po:po + 64, to:to + ts],
                                             QT[c2][po:po + 64, lo:lo + ls],
                                             start=True, stop=True)
                        et = etp.tile([128, 2048], BF16, tag="et", name="et")
                        wid = (len(tt) - 1) * 512 + ls
                        nc.scalar.activation(et[:, 0:wid], p[:, 0:wid],
                                             AFT.Exp, scale=float(SCALE))
                        ets.append(et)
                    po2 = psO.tile([65, 512], F32, tag="psO", name="psO")
                    for ti, (to, ts) in enumerate(T_TILES):
                        nc.tensor.matmul(
                            po2[:, :ls], Vh[ti][0:ts, h * 65:(h + 1) * 65],
                            ets[ti // 4][0:ts, (ti % 4) * 512:(ti % 4) * 512 + ls],
                            start=(ti == 0), stop=(ti == 6))
                    nc.vector.tensor_copy(
                        OTb[po:po + 64, c2 * 512:c2 * 512 + ls], po2[0:64, :ls])
                    nc.vector.tensor_copy(rcf[0:1, h * 512:h * 512 + ls],
                                          po2[64:65, :ls])
                # gather sums -> 6 partitions, recip, indicator bcast, scale
                rin = rcf[0:1, :].rearrange("p (g l) -> p g l", l=512)
                nc.sync.dma_start(rc6[0:6, 0:ls], rin[:, :, 0:ls])
                with nc.allow_low_precision(reason="f32r recip"):
                    nc.vector.reciprocal(rc6r[0:6, 0:ls], rc6[0:6, 0:ls])
                if "dbg_rc6" in d and lo == 0:
                    nc.sync.dma_start(d["dbg_rc6"], rc6[:])
                rbp = psS.tile([128, 2048], F32, tag="psS", name="psS")
                for ch in range(3):
                    nc.tensor.matmul(rbp[:, ch * 512:ch * 512 + ls],
                                     ind6[ch][:], rc6r[0:6, 0:ls],
                                     start=True, stop=True)
                o3 = OTb[:].rearrange("p (w l) -> p w l", l=512)[:, :, 0:ls]
                r3 = rbp[:].rearrange("p (w l) -> p w l", l=512)[:, 0:3, 0:ls]
                nc.vector.tensor_mul(o3, o3, r3)
                if "dbg_otb" in d and lo == 0:
                    nc.sync.dma_start(d["dbg_otb"], OTb[:])
                # output projection: [l, co] layout, 4 l-tile windows
                p = psS.tile([128, 2048], F32, tag="psS", name="psS")
                nlt = (ls + 127) // 128
                for k in range(nlt):
                    lsz = min(128, ls - k * 128)
                    win = p[0:lsz, k * 512:k * 512 + C]
                    nc.tensor.matmul(win, ones1[0:1, 0:lsz], bpjR[0:1, :],
                                     start=True, stop=False)
                    for ch in range(3):
                        nc.tensor.matmul(
                            win, OTb[:, ch * 512 + k * 128:
                                     ch * 512 + k * 128 + lsz],
                            wpj[ch][:], start=False, stop=(ch == 2))
                osb = outp.tile([128, 2048], F32, tag="o", name="o")
                ov = osb[:].rearrange("p (w c) -> p w c", c=512)[:, 0:nlt, 0:C]
                pv = p[:].rearrange("p (w c) -> p w c", c=512)[:, 0:nlt, 0:C]
                nc.vector.tensor_copy(ov, pv)
                lsz = min(128, ls - (nlt - 1) * 128)
                if lsz < 128:
                    ov = ov[0:lsz]
                dst = d["out"][lo:lo + ls, :].rearrange(
                    "(w p) c -> p w c", p=min(128, ls))
                nc.sync.dma_start(dst, ov)


def _build(reps=1):
    if reps in _CACHE:
        return _CACHE[reps]
    nc = bacc.Bacc("TRN2", target_bir_lowering=False, debug=False)
    d = {
        "xp": nc.dram_tensor("xp", [C, XP], F32, kind="ExternalInput").ap(),
        "wb": nc.dram_tensor("wb", [3, 128, 30], F32, kind="ExternalInput").ap(),
        "wq": nc.dram_tensor("wq", [C, C], BF16, kind="ExternalInput").ap(),
        "wk": nc.dram_tensor("wk", [C, C], BF16, kind="ExternalInput").ap(),
        "wvp": nc.dram_tensor("wvp", [C, NH * 65], BF16,
                              kind="ExternalInput").ap(),
        "wpj": nc.dram_tensor("wpj", [C, C], BF16, kind="ExternalInput").ap(),
        "ind6": nc.dram_tensor("ind6", [3, 6, 128], F32R,
                               kind="ExternalInput").ap(),
        "bpjR": nc.dram_tensor("bpjR", [1, C], F32R, kind="ExternalInput").ap(),
        "out": nc.dram_tensor("out", [T, C], F32, kind="ExternalOutput").ap(),
    }
    with tile.TileContext(nc) as tc:
        with contextlib.ExitStack() as ctx:
            _emit(nc, tc, ctx, d, reps)
    nc.compile()
    _CACHE[reps] = nc
    return nc


def _host_prep(x, conv_q, conv_k, conv_v, bn_q, bn_k, bn_v, Wq, Wk, Wv,
               Wproj, bproj):
    B = x.shape[0]
    x = np.asarray(x, np.float32)
    xp = np.zeros((B, C, XP), np.float32)
    xp[:, :, 56:56 + T] = np.ascontiguousarray(x.transpose(0, 2, 1))

    wb = np.zeros((3, 128, 30), np.float32)
    for cv, (w, bn) in enumerate(((conv_q, bn_q), (conv_k, bn_k),
                                  (conv_v, bn_v))):
        g, b, m, v = [np.asarray(bn[i], np.float64) for i in range(4)]
        a = g / np.sqrt(v + EPS)
        bias = (b - m * a).astype(np.float32)
        wh = (np.asarray(w, np.float64).reshape(C, 9) * a[:, None]).astype(
            np.float32)
        for ch in range(3):
            wb[ch, :, 9 * cv:9 * cv + 9] = wh[ch * 128:(ch + 1) * 128]
            wb[ch, :, 27 + cv] = bias[ch * 128:(ch + 1) * 128]

    wvp = np.zeros((C, NH * 65), np.float32)
    Wv = np.asarray(Wv, np.float32)
    for h in range(NH):
        wvp[:, h * 65:h * 65 + 64] = Wv[:, h * 64:(h + 1) * 64]

    ind6 = np.zeros((3, 6, 128), np.float32)
    for ch in range(3):
        ind6[ch, 2 * ch, 0:64] = 1.0
        ind6[ch, 2 * ch + 1, 64:128] = 1.0

    bf = ml_dtypes.bfloat16
    return {
        "xp": xp,
        "wb": wb,
        "wq": np.asarray(Wq, np.float32).astype(bf),
        "wk": np.asarray(Wk, np.float32).astype(bf),
        "wvp": wvp.astype(bf),
        "wpj": np.asarray(Wproj, np.float32).astype(bf),
        "ind6": ind6,
        "bpjR": np.asarray(bproj, np.float32).reshape(1, C),
    }


def kernel(x, h, w, conv_q, conv_k, conv_v, bn_q, bn_k, bn_v, Wq, Wk, Wv,
           Wproj, bproj, _reps=1, _nc=None):
    B = x.shape[0]
    nc = _nc if _nc is not None else _build(_reps)
    hp = _host_prep(x, conv_q, conv_k, conv_v, bn_q, bn_k, bn_v, Wq, Wk, Wv,
                    Wproj, bproj)
    shared = {k: v for k, v in hp.items() if k != "xp"}
    in_maps = [dict(shared, xp=hp["xp"][b]) for b in range(B)]
    res = run_bass_kernel_spmd(nc, in_maps, core_ids=list(range(B)))
    out = np.stack([res.results[b]["out"] for b in range(B)], axis=0)
    return out.astype(np.float32)
